# revision 20
# baseline (speedup 1.0000x reference)
"""Distributed KNN retrieval (top-2 over a 1M-column L1-normalized bank) on 8 trn2 cores.

Strategy: shard the active bank columns ([0,start) u [end,N)) evenly across 8
cores, shipped as bf16 (halves HBM traffic; ranking tolerates it).  Each core
computes sim^T = bank_chunk.T @ qT on the tensor engine with the bank chunk
as the (bf16, FWL-accelerated) stationary operand, so PSUM holds
[128 bank-cols, 16 queries] — the reductions then run at full 128-partition
width.  PSUM blocks are copied into a big SBUF buffer S[128, 16*C]
(query-major) by the scalar engine; per query one vector-engine max (top-8)
+ max_index pass over the 928 chunk-slots yields per-(partition, query)
top-8 candidates.  The host reduces the 8*128*8 candidates per query to a
top-16 shortlist, recomputes those dot products exactly in f32, and picks
the final top-2 with jax-compatible tie-breaking — so the returned
values/indices are exact even though the device ranking ran in bf16.
"""

import sys

for _p in ("/opt/trn_rl_repo",):
    if _p not in sys.path:
        sys.path.insert(0, _p)

import numpy as np

Q = 16
D = 384
TOPK = 2
EPS = 1e-12
N_CORES = 8
P = 128
KP = D // P  # 3 contraction chunks
CHUNKS_PER_SLAB = 16  # 2048 cols per DMA slab
BLK = 32  # chunks accumulated per PSUM bank before copy-out

TRACE = False
LAST_EXEC_NS = None
LAST_RESULTS = None

_BUILD_CACHE = {}


def _build_program(chunks_per_core):
    import concourse.bacc as bacc
    import concourse.mybir as mybir
    import concourse.tile as tile

    C = chunks_per_core
    per_core = C * P
    ncols = Q + per_core  # first Q cols of the input carry qT
    f32 = mybir.dt.float32
    bf16 = mybir.dt.bfloat16
    u32 = mybir.dt.uint32

    nc = bacc.Bacc()
    bank_d = nc.dram_tensor("bank", [D, ncols], bf16, kind="ExternalInput")
    vals_d = nc.dram_tensor("vals8", [P, Q * 8], f32, kind="ExternalOutput")
    idx_d = nc.dram_tensor("idx8", [P, Q * 8], u32, kind="ExternalOutput")

    SLAB = CHUNKS_PER_SLAB * P  # bank cols per range

    with tile.TileContext(nc) as tc:
        with (
            tc.tile_pool(name="slab0", bufs=1) as slab0_pool,
            tc.tile_pool(name="slab", bufs=4) as slab_pool,
            tc.tile_pool(name="spool", bufs=1) as s_pool,
            tc.tile_pool(name="psum", bufs=8, space="PSUM") as psum_pool,
            tc.tile_pool(name="outp", bufs=1) as out_pool,
        ):
            bank3 = bank_d[:, :].rearrange("(k p) n -> p k n", k=KP)  # [128,3,ncols]

            S = s_pool.tile([P, Q * C], f32)  # col = q*C + c (query-major)
            S3 = S[:, :].rearrange("p (q c) -> p q c", q=Q)

            # range 0: qT + first SLAB cols in ONE dma, into a dedicated tile
            w0 = Q + SLAB
            slab0 = slab0_pool.tile([P, KP * w0], bf16)
            nc.sync.dma_start(
                out=slab0[:, :].rearrange("p (k c) -> p k c", k=KP),
                in_=bank3[:, :, 0:w0],
            )

            def qt_ap(k):
                return slab0[:, k * w0 : k * w0 + Q]

            ps = None
            blk_start = 0
            for r in range(C // CHUNKS_PER_SLAB):
                c0 = r * CHUNKS_PER_SLAB
                if r == 0:
                    cur, base, kw = slab0, Q, w0
                else:
                    cur = slab_pool.tile([P, KP * SLAB], bf16, tag="slab")
                    nc.sync.dma_start(
                        out=cur[:, :].rearrange("p (k c) -> p k c", k=KP),
                        in_=bank3[:, :, Q + c0 * P : Q + c0 * P + SLAB],
                    )
                    base, kw = 0, SLAB
                for g in range(CHUNKS_PER_SLAB):
                    c = c0 + g
                    if c % BLK == 0:
                        ps = psum_pool.tile([P, BLK * Q], f32)
                        blk_start = c
                    j = c - blk_start
                    for k in range(KP):
                        off = k * kw + base + g * P
                        nc.tensor.matmul(
                            ps[:, j * Q : (j + 1) * Q],
                            lhsT=cur[:, off : off + P],
                            rhs=qt_ap(k),
                            start=(k == 0),
                            stop=(k == KP - 1),
                        )
                    nxt = c + 1
                    if nxt == C or nxt % BLK == 0:
                        nblk = nxt - blk_start
                        src = ps[:, : nblk * Q].rearrange("p (b q) -> p b q", b=nblk)
                        dst = S3[:, :, blk_start:nxt].transpose([0, 2, 1])
                        nc.scalar.copy(out=dst, in_=src)

            vals8 = out_pool.tile([P, Q * 8], f32)
            idx8 = out_pool.tile([P, Q * 8], u32)
            for q in range(Q):
                nc.vector.max(
                    out=vals8[:, q * 8 : (q + 1) * 8], in_=S[:, q * C : (q + 1) * C]
                )
                nc.vector.max_index(
                    out=idx8[:, q * 8 : (q + 1) * 8],
                    in_max=vals8[:, q * 8 : (q + 1) * 8],
                    in_values=S[:, q * C : (q + 1) * C],
                )
            nc.sync.dma_start(out=vals_d[:, :], in_=vals8[:, :])
            nc.sync.dma_start(out=idx_d[:, :], in_=idx8[:, :])

    nc.finalize()
    return nc


def _get_program(chunks_per_core):
    if chunks_per_core not in _BUILD_CACHE:
        _BUILD_CACHE[chunks_per_core] = _build_program(chunks_per_core)
    return _BUILD_CACHE[chunks_per_core]


def _to_bf16_bits(x):
    """Round-to-nearest-even f32 -> bf16, as a uint16 array (fast path)."""
    u = x.view(np.uint32)
    r = ((u + 0x7FFF + ((u >> 16) & 1)) >> 16).astype(np.uint16)
    return r


def _build_shards(bank_bits, qt_bits, start, end, per_core, n_bank):
    """Per-core [D, Q + per_core] bf16-bit arrays: qT in the first Q cols,
    then a contiguous slice of the active column set (zero-padded tail)."""
    gap = end - start
    active_n = n_bank - gap
    shards = []
    for i in range(N_CORES):
        lo = i * per_core
        hi = min(lo + per_core, active_n)
        shard = np.zeros((D, Q + per_core), dtype=np.uint16)
        shard[:, :Q] = qt_bits
        if hi > lo:
            if hi <= start:
                shard[:, Q : Q + hi - lo] = bank_bits[:, lo:hi]
            elif lo >= start:
                shard[:, Q : Q + hi - lo] = bank_bits[:, lo + gap : hi + gap]
            else:
                shard[:, Q : Q + start - lo] = bank_bits[:, lo:start]
                shard[:, Q + start - lo : Q + hi - lo] = bank_bits[:, end : hi + gap]
        shards.append(shard)
    return shards, active_n


def kernel(**inputs):
    global LAST_EXEC_NS, LAST_RESULTS
    import ml_dtypes
    from concourse.bass_utils import run_bass_kernel_spmd

    query_emb = np.asarray(inputs["query_emb"], dtype=np.float32)
    bank = np.asarray(inputs["bank"], dtype=np.float32)
    start = int(inputs["start"])
    end = int(inputs["end"])
    gap = end - start

    active_n = bank.shape[1] - gap
    # chunks per core, rounded up so per-core cols divide into DMA slabs
    C = -(-active_n // (N_CORES * CHUNKS_PER_SLAB * P)) * CHUNKS_PER_SLAB
    per_core = C * P

    # L1-normalize queries on host (24KB of work) and transpose to [D, Q]
    qn = query_emb / np.clip(
        np.sum(np.abs(query_emb), axis=1, keepdims=True), EPS, None
    )
    qt = np.ascontiguousarray(qn.T.astype(np.float32))

    bank_bits = _to_bf16_bits(bank)
    qt_bits = _to_bf16_bits(qt)
    shards, active_n = _build_shards(
        bank_bits, qt_bits, start, end, per_core, bank.shape[1]
    )
    nc = _get_program(C)

    in_maps = [{"bank": shards[i].view(ml_dtypes.bfloat16)} for i in range(N_CORES)]
    kw = {}
    if TRACE:
        kw = dict(trace=True, trace_cores=list(range(N_CORES)))
    res = run_bass_kernel_spmd(nc, in_maps, list(range(N_CORES)), **kw)
    LAST_EXEC_NS = res.exec_time_ns
    LAST_RESULTS = res

    vals = np.stack([r["vals8"] for r in res.results])  # [8, P, Q*8]
    cidx = np.stack([r["idx8"] for r in res.results])  # [8, P, Q*8] uint32

    vals = vals.reshape(N_CORES, P, Q, 8)
    cidx = cidx.reshape(N_CORES, P, Q, 8).astype(np.int64)

    core = np.arange(N_CORES)[:, None, None, None]
    part = np.arange(P)[None, :, None, None]
    a = core * per_core + cidx * P + part  # active-set index
    valid = a < active_n
    orig = a + np.where(a >= start, gap, 0)

    # flatten candidates per query
    v = np.moveaxis(vals, 2, 0).reshape(Q, -1).astype(np.float32)
    o = np.moveaxis(orig, 2, 0).reshape(Q, -1)
    m = np.moveaxis(valid, 2, 0).reshape(Q, -1)
    v = np.where(m, v, -np.inf)

    # device sims are bf16-ranked; re-rank a top-16 shortlist per query with
    # exact f32 dot products on host before the final top-2
    R = 16
    out_vals = np.empty((Q, TOPK), dtype=np.float32)
    out_idx = np.empty((Q, TOPK), dtype=np.int32)
    for q in range(Q):
        order = np.lexsort((o[q], -v[q]))[:R]
        cand = o[q][order]
        exact = qn[q].astype(np.float32) @ bank[:, cand].astype(np.float32)
        pick = np.lexsort((cand, -exact))[:TOPK]
        out_vals[q] = exact[pick]
        out_idx[q] = cand[pick]
    return out_vals, out_idx
